# revision 2
# baseline (speedup 1.0000x reference)
"""Sparse Conv3d (3x3x3, torchsparse kmap) + BatchNorm + ReLU on 8 TRN2 NeuronCores.

Strategy (voxel/data parallel, per sharding hint):
  - Output voxels sharded across 8 cores (15000 rows each).
  - feats replicated in DRAM as 4 banks of 30000 rows (+1 zero row each) so
    gather indices fit int16 for the HW dma_gather path.
  - Center offset (k=13) is the identity map: its matmul operand is loaded
    directly from a per-core transposed feats slice (no gather).
  - Off-center offsets: host-compacted valid (src,dst) pairs; device does
    dma_gather (bank-major) -> PE transpose -> K=64 matmul with W_k ->
    dma_scatter_add into an SBUF accumulator (parity-split even/odd slots).
    Scatter calls are per-offset so destinations are unique within a call
    (duplicate dsts within one call lose updates on HW; across calls they
    accumulate correctly).
  - BN stats via PE matmuls (ones^T @ X for sums, X^T X diag for sum-squares),
    AllReduce [1,128] across the 8 cores, normalize + ReLU on-chip, output in
    a row-wrapped layout that the host unwraps.
"""

import sys
import os

for _p in ("/opt/trn_rl_repo", "/root/.axon_site/_ro/trn_rl_repo"):
    if os.path.isdir(_p) and _p not in sys.path:
        sys.path.insert(0, _p)

import numpy as np

N = 120000
CIN = 64
COUT = 64
K = 27
CENTER = 13
EPS = 1e-5
NCORES = 8
NC_ROWS = N // NCORES          # 15000
BANK = 30000
NBANK = 4
ZROW = BANK                     # zero row index within a bank table
SLOTS = 118                     # ceil(15000/128); wrapped rows = 15104
WRAP_ROWS = SLOTS * 128         # 15104
TRASH = WRAP_ROWS - 1           # trash dst row 15103 (slot 117, partition 127)
HGRP = (SLOTS + 1) // 2         # 59 groups per parity


def _wrap16(idx):
    """Wrap an int stream into the [128, n/16] int16 layout dma_gather expects."""
    n = len(idx)
    assert n % 16 == 0
    w = np.ascontiguousarray(idx.reshape(n // 16, 16).T).astype(np.int16)
    return np.tile(w, (8, 1))


def _plan(nbr):
    """Host-side index preprocessing. Returns static chunk metadata (shared
    across cores) and per-core gather/scatter index tensors."""
    offs = [k for k in range(K) if k != CENTER]
    counts = np.zeros((NCORES, K, NBANK), np.int64)
    lists = {}
    for k in offs:
        v = nbr[k]
        for c in range(NCORES):
            seg = v[c * NC_ROWS:(c + 1) * NC_ROWS]
            val = np.nonzero(seg >= 0)[0]
            src = seg[val].astype(np.int64)
            bank = src // BANK
            for b in range(NBANK):
                m = bank == b
                lists[(c, k, b)] = (src[m] - b * BANK, val[m])
                counts[c, k, b] = int(m.sum())
    ckb = -(-counts.max(axis=0) // 128)  # [K, NBANK] chunks, 0 for CENTER row
    ckb[CENTER, :] = 0

    # gather stream: bank-major, offsets ascending inside each bank
    gbase = np.zeros(NBANK, np.int64)   # slot base per bank
    gslot = {}                          # (k, b) -> first gather slot (global)
    pos = 0
    sg_b = []
    for b in range(NBANK):
        gbase[b] = pos
        for k in offs:
            if ckb[k, b]:
                gslot[(k, b)] = pos
                pos += ckb[k, b]
        sg_b.append(pos - gbase[b])
    sg_total = pos

    # scatter stream: offset-major; chunk (k, b, j) -> scatter slot
    spos = {}
    p = 0
    ck_tot = {}
    for k in offs:
        ck = int(ckb[k].sum())
        ck_tot[k] = ck
        spos[k] = p
        p += ck
    ss_total = p

    gidx_cores, sidx_cores = [], []
    for c in range(NCORES):
        gstream = np.full(sg_total * 128, ZROW, np.int64)
        for b in range(NBANK):
            for k in offs:
                if not ckb[k, b]:
                    continue
                loc, _ = lists[(c, k, b)]
                s0 = gslot[(k, b)] * 128
                gstream[s0:s0 + len(loc)] = loc
        sstream = np.full(ss_total * 128, TRASH, np.int64)
        for k in offs:
            base = spos[k] * 128
            o = 0
            for b in range(NBANK):
                if not ckb[k, b]:
                    continue
                _, dst = lists[(c, k, b)]
                sstream[base + o:base + o + len(dst)] = dst
                o += ckb[k, b] * 128
        # wrap per call: gather call = per bank; scatter call = per offset
        gw = [
            _wrap16(gstream[gbase[b] * 128:(gbase[b] + sg_b[b]) * 128])
            for b in range(NBANK) if sg_b[b]
        ]
        sw = [
            _wrap16(sstream[spos[k] * 128:(spos[k] + ck_tot[k]) * 128])
            for k in offs if ck_tot[k]
        ]
        gidx_cores.append(np.concatenate(gw, axis=1))
        sidx_cores.append(np.concatenate(sw, axis=1))

    meta = dict(offs=offs, ckb=ckb, gbase=gbase, sg_b=sg_b, sg_total=sg_total,
                gslot=gslot, spos=spos, ck_tot=ck_tot, ss_total=ss_total)
    return meta, gidx_cores, sidx_cores


def _build_bass(meta):
    from concourse import mybir, bacc
    import concourse.tile as tile
    from concourse.masks import make_identity

    offs = meta["offs"]
    ckb = meta["ckb"]
    gbase = meta["gbase"]
    sg_b = meta["sg_b"]
    gslot = meta["gslot"]
    ck_tot = meta["ck_tot"]
    sg_total = meta["sg_total"]
    ss_total = meta["ss_total"]
    f32 = mybir.dt.float32
    i16 = mybir.dt.int16

    nc = bacc.Bacc("TRN2", target_bir_lowering=False, debug=False,
                   num_devices=NCORES)
    feats4 = nc.dram_tensor("feats4", [NBANK * (BANK + 1), CIN], f32,
                            kind="ExternalInput").ap()
    wmat = nc.dram_tensor("wmat", [CIN, K * COUT], f32,
                          kind="ExternalInput").ap()
    ftc = nc.dram_tensor("ftc", [CIN, WRAP_ROWS], f32,
                         kind="ExternalInput").ap()
    gidx = nc.dram_tensor("gidx", [128, sg_total * 8], i16,
                          kind="ExternalInput").ap()
    sixd = nc.dram_tensor("sixd", [128, ss_total * 8], i16,
                          kind="ExternalInput").ap()
    gbeta = nc.dram_tensor("gbeta", [1, 128], f32, kind="ExternalInput").ap()
    oute = nc.dram_tensor("oute", [128, HGRP, COUT], f32,
                          kind="ExternalOutput").ap()
    outo = nc.dram_tensor("outo", [128, HGRP, COUT], f32,
                          kind="ExternalOutput").ap()

    with tile.TileContext(nc) as tc:
        with tc.tile_pool(name="sb", bufs=1) as pool, \
             tc.tile_pool(name="ps", bufs=2, space="PSUM") as psum, \
             tc.tile_pool(name="dram", bufs=1, space="DRAM") as dram:
            ident = pool.tile([128, 128], f32)
            make_identity(nc, ident[:])
            ones = pool.tile([128, 1], f32)
            nc.vector.memset(ones[:], 1.0)
            onesr = pool.tile([1, 128], f32)
            nc.vector.memset(onesr[:], 1.0)
            istack = pool.tile([128, COUT], f32)
            nc.vector.tensor_copy(out=istack[0:64, :], in_=ident[0:64, 0:64])
            nc.vector.tensor_copy(out=istack[64:128, :],
                                  in_=ident[64:128, 64:128])

            gix = pool.tile([128, sg_total * 8], i16)
            nc.sync.dma_start(out=gix[:], in_=gidx[:])
            six = pool.tile([128, ss_total * 8], i16)
            nc.sync.dma_start(out=six[:], in_=sixd[:])
            wsb = pool.tile([CIN, K * COUT], f32)
            nc.sync.dma_start(out=wsb[:], in_=wmat[:])
            gb = pool.tile([1, 128], f32)
            nc.sync.dma_start(out=gb[:], in_=gbeta[:])

            # accumulators (SBUF resident), initialized by the center pass
            ae = pool.tile([128, HGRP, COUT], f32)
            ao = pool.tile([128, HGRP, COUT], f32)

            # ---- gathers (4 calls, bank-major stream) ----
            gtiles = {}
            for b in range(NBANK):
                if not sg_b[b]:
                    continue
                g = pool.tile([128, sg_b[b], CIN], f32, tag=f"g{b}")
                gtiles[b] = g
                nc.gpsimd.dma_gather(
                    out_ap=g[:],
                    in_ap=feats4[(BANK + 1) * b:(BANK + 1) * (b + 1), :],
                    idxs_ap=gix[:, gbase[b] * 8:(gbase[b] + sg_b[b]) * 8],
                    num_idxs=sg_b[b] * 128, num_idxs_reg=sg_b[b] * 128,
                    elem_size=CIN, single_packet=False)

            # ---- center pass: ftc slices -> matmul -> init ae/ao ----
            piece_chunks = [30, 30, 30, 28]
            wc = wsb[:, CENTER * COUT:(CENTER + 1) * COUT]
            jglob = 0
            for pc_i, pch in enumerate(piece_chunks):
                fpc = pool.tile([CIN, pch * 128], f32, tag="ftc", bufs=2)
                nc.sync.dma_start(
                    out=fpc[:], in_=ftc[:, jglob * 128:(jglob + pch) * 128])
                for j0 in range(0, pch, 8):
                    jn = min(8, pch - j0)
                    pe = psum.tile([128, 4, COUT], f32, tag="pcE")
                    po = psum.tile([128, 4, COUT], f32, tag="pcO")
                    ne = no = 0
                    for j in range(j0, j0 + jn):
                        gj = jglob + j
                        lhsT = fpc[:, j * 128:(j + 1) * 128]
                        if gj % 2 == 0:
                            out_ap = pe[:, ne, :]; ne += 1
                        else:
                            out_ap = po[:, no, :]; no += 1
                        nc.tensor.matmul(out=out_ap, lhsT=lhsT, rhs=wc,
                                         start=True, stop=True)
                    ge0 = (jglob + j0 + 1) // 2
                    go0 = (jglob + j0) // 2
                    if (jglob + j0) % 2 == 0:
                        ge0 = (jglob + j0) // 2
                        go0 = (jglob + j0) // 2
                    if ne:
                        nc.vector.tensor_copy(out=ae[:, ge0:ge0 + ne, :],
                                              in_=pe[:, :ne, :])
                    if no:
                        nc.vector.tensor_copy(out=ao[:, go0:go0 + no, :],
                                              in_=po[:, :no, :])
                jglob += pch

            # ---- off-center: transpose -> W_k matmul -> scatter-add ----
            for k in offs:
                ck = ck_tot[k]
                if not ck:
                    continue
                y = pool.tile([128, ck, COUT], f32, tag="y", bufs=3)
                wk = wsb[:, k * COUT:(k + 1) * COUT]
                # chunk list for this offset in scatter order
                chunks = []
                for b in range(NBANK):
                    for j in range(ckb[k, b]):
                        chunks.append(gslot[(k, b)] - gbase[b] + j
                                      + (b << 20))  # encode bank
                for i0 in range(0, ck, 8):
                    inb = min(8, ck - i0)
                    py = psum.tile([128, 8, COUT], f32, tag="py")
                    for q in range(inb):
                        enc = chunks[i0 + q]
                        b, slot = enc >> 20, enc & ((1 << 20) - 1)
                        pt = psum.tile([CIN, 128], f32, tag="pt", bufs=2)
                        nc.tensor.transpose(out=pt[:],
                                            in_=gtiles[b][:, slot, :],
                                            identity=ident[:])
                        gt = pool.tile([CIN, 128], f32, tag="gt", bufs=4)
                        nc.vector.tensor_copy(out=gt[:], in_=pt[:])
                        nc.tensor.matmul(out=py[:, q, :], lhsT=gt[:], rhs=wk,
                                         start=True, stop=True)
                    nc.vector.tensor_copy(out=y[:, i0:i0 + inb, :],
                                          in_=py[:, :inb, :])
                nc.gpsimd.dma_scatter_add(
                    out_ap=ae[:], in_ap=y[:, :, :],
                    idxs_ap=six[:, meta["spos"][k] * 8:(meta["spos"][k] + ck) * 8],
                    num_idxs=ck * 128, num_idxs_reg=ck * 128, elem_size=COUT,
                    sbuf_tokens_per_rank=128, parity_reg=0, out_ap_other=ao[:],
                    single_packet=False)

            # ---- zero the trash region (rows 15072..15103 incl. TRASH) ----
            # other pad rows (15000..15071) only ever receive center zeros
            nc.vector.memset(ao[96:128, 58, :], 0.0)

            # ---- stats: sums + sum-squares over all rows ----
            pcov = psum.tile([128, 128], f32, tag="py")
            cov_ins = []
            for t in (ae, ao):
                for g0 in range(0, HGRP - 1, 2):
                    cov_ins.append(t[:, g0:g0 + 2, :])
                cov_ins.append(t[:, HGRP - 1:HGRP, :])
            for i, ap in enumerate(cov_ins):
                w = ap.shape[1] * COUT
                nc.tensor.matmul(out=pcov[0:w, 0:w], lhsT=ap, rhs=ap,
                                 start=(i == 0), stop=(i == len(cov_ins) - 1))
            psumr = psum.tile([1, 512], f32, tag="pcE")
            sum_ins = []
            for t in (ae, ao):
                for g0 in range(0, HGRP, 8):
                    gn = min(8, HGRP - g0)
                    sum_ins.append(t[:, g0:g0 + gn, :])
            for i, ap in enumerate(sum_ins):
                w = ap.shape[1] * COUT
                nc.tensor.matmul(out=psumr[:, 0:w], lhsT=ones[:], rhs=ap,
                                 start=(i == 0), stop=(i == len(sum_ins) - 1))
            tmpc = pool.tile([128, 128], f32)
            nc.vector.tensor_mul(out=tmpc[:], in0=pcov[:], in1=ident[:])
            diagc = pool.tile([128, 1], f32)
            nc.vector.tensor_reduce(out=diagc[:], in_=tmpc[:],
                                    axis=mybir.AxisListType.X,
                                    op=mybir.AluOpType.add)
            psq = psum.tile([1, COUT], f32, tag="pt")
            nc.tensor.matmul(out=psq[:], lhsT=diagc[:], rhs=istack[:],
                             start=True, stop=True)
            ssum = pool.tile([1, 512], f32)
            nc.vector.tensor_copy(out=ssum[:], in_=psumr[:])
            nc.vector.tensor_add(out=ssum[:, 0:256], in0=ssum[:, 0:256],
                                 in1=ssum[:, 256:512])
            nc.vector.tensor_add(out=ssum[:, 0:128], in0=ssum[:, 0:128],
                                 in1=ssum[:, 128:256])
            nc.vector.tensor_add(out=ssum[:, 0:64], in0=ssum[:, 0:64],
                                 in1=ssum[:, 64:128])
            stats = pool.tile([1, 128], f32)
            nc.vector.tensor_copy(out=stats[:, 0:64], in_=ssum[:, 0:64])
            nc.vector.tensor_copy(out=stats[:, 64:128], in_=psq[:])

            # ---- AllReduce over 8 cores ----
            cin_d = dram.tile([1, 128], f32)
            cout_d = dram.tile([1, 128], f32)
            nc.sync.dma_start(out=cin_d[:], in_=stats[:])
            if os.environ.get("BASS_SIM_NO_COLLECTIVE"):
                nc.sync.dma_start(out=cout_d[:], in_=cin_d[:])
            else:
                nc.gpsimd.collective_compute(
                    "AllReduce", mybir.AluOpType.add,
                    replica_groups=[list(range(NCORES))],
                    ins=[cin_d.opt()], outs=[cout_d.opt()])
            red = pool.tile([1, 128], f32)
            nc.sync.dma_start(out=red[:], in_=cout_d[:])

            # ---- affine params ----
            mean = pool.tile([1, COUT], f32)
            nc.vector.tensor_scalar_mul(out=mean[:], in0=red[:, 0:64],
                                        scalar1=1.0 / N)
            ex2 = pool.tile([1, COUT], f32)
            nc.vector.tensor_scalar_mul(out=ex2[:], in0=red[:, 64:128],
                                        scalar1=1.0 / N)
            var = pool.tile([1, COUT], f32)
            nc.vector.tensor_mul(out=var[:], in0=mean[:], in1=mean[:])
            nc.vector.tensor_sub(out=var[:], in0=ex2[:], in1=var[:])
            nc.vector.tensor_scalar_add(out=var[:], in0=var[:], scalar1=EPS)
            std = pool.tile([1, COUT], f32)
            nc.scalar.sqrt(out=std[:], in_=var[:])
            rstd = pool.tile([1, COUT], f32)
            nc.vector.reciprocal(out=rstd[:], in_=std[:])
            scl = pool.tile([1, COUT], f32)
            nc.vector.tensor_mul(out=scl[:], in0=gb[:, 0:64], in1=rstd[:])
            bia = pool.tile([1, COUT], f32)
            nc.vector.tensor_mul(out=bia[:], in0=mean[:], in1=scl[:])
            nc.vector.tensor_sub(out=bia[:], in0=gb[:, 64:128], in1=bia[:])

            # broadcast to [128, 8, 64] (pattern repeats every 64 cols)
            pbs = psum.tile([128, COUT], f32, tag="pt")
            nc.tensor.matmul(out=pbs[:], lhsT=onesr[:], rhs=scl[:],
                             start=True, stop=True)
            s8 = pool.tile([128, 8, COUT], f32)
            nc.vector.tensor_copy(out=s8[:, 0, :], in_=pbs[:])
            pbb = psum.tile([128, COUT], f32, tag="pt")
            nc.tensor.matmul(out=pbb[:], lhsT=onesr[:], rhs=bia[:],
                             start=True, stop=True)
            b8 = pool.tile([128, 8, COUT], f32)
            nc.vector.tensor_copy(out=b8[:, 0, :], in_=pbb[:])
            for t8 in (s8, b8):
                nc.vector.tensor_copy(out=t8[:, 1:2, :], in_=t8[:, 0:1, :])
                nc.vector.tensor_copy(out=t8[:, 2:4, :], in_=t8[:, 0:2, :])
                nc.vector.tensor_copy(out=t8[:, 4:8, :], in_=t8[:, 0:4, :])

            # ---- normalize + relu in place, then write out ----
            for t in (ae, ao):
                for g0 in range(0, HGRP, 8):
                    gn = min(8, HGRP - g0)
                    sl = t[:, g0:g0 + gn, :]
                    nc.vector.tensor_mul(out=sl, in0=sl, in1=s8[:, 0:gn, :])
                    nc.vector.tensor_add(out=sl, in0=sl, in1=b8[:, 0:gn, :])
                    nc.vector.tensor_scalar_max(out=sl, in0=sl, scalar1=0.0)
            nc.sync.dma_start(out=oute[:], in_=ae[:, :, :])
            nc.sync.dma_start(out=outo[:], in_=ao[:, :, :])

    nc.compile()
    return nc


def _host_tensors(feats, weight, gamma, beta):
    feats = np.ascontiguousarray(np.asarray(feats, dtype=np.float32))
    weight = np.asarray(weight, dtype=np.float32)
    f4 = np.zeros((NBANK * (BANK + 1), CIN), np.float32)
    for b in range(NBANK):
        f4[b * (BANK + 1):b * (BANK + 1) + BANK] = \
            feats[b * BANK:(b + 1) * BANK]
    wm = np.ascontiguousarray(
        weight.transpose(1, 0, 2).reshape(CIN, K * COUT))
    gbv = np.zeros((1, 128), np.float32)
    gbv[0, 0:64] = np.asarray(gamma, np.float32)
    gbv[0, 64:128] = np.asarray(beta, np.float32)
    ftcs = []
    for c in range(NCORES):
        t = np.zeros((CIN, WRAP_ROWS), np.float32)
        t[:, :NC_ROWS] = feats[c * NC_ROWS:(c + 1) * NC_ROWS].T
        ftcs.append(t)
    return f4, wm, gbv, ftcs


def kernel(feats, weight, gamma, beta, neighbor_idx):
    from concourse.bass_utils import run_bass_kernel_spmd

    nbr = np.asarray(neighbor_idx)
    meta, gidx_cores, sidx_cores = _plan(nbr)
    nc = _build_bass(meta)
    f4, wm, gbv, ftcs = _host_tensors(feats, weight, gamma, beta)
    in_maps = [
        {"feats4": f4, "wmat": wm, "ftc": ftcs[c], "gidx": gidx_cores[c],
         "sixd": sidx_cores[c], "gbeta": gbv}
        for c in range(NCORES)
    ]
    res = run_bass_kernel_spmd(nc, in_maps, core_ids=list(range(NCORES)))
    out = np.empty((N, COUT), np.float32)
    for c in range(NCORES):
        wrapped = np.empty((128, SLOTS, COUT), np.float32)
        wrapped[:, 0::2, :] = res.results[c]["oute"]
        wrapped[:, 1::2, :] = res.results[c]["outo"]
        rows = wrapped.transpose(1, 0, 2).reshape(WRAP_ROWS, COUT)
        out[c * NC_ROWS:(c + 1) * NC_ROWS] = rows[:NC_ROWS]
    return out



# revision 3
# speedup vs baseline: 10.7401x; 10.7401x over previous
"""Sparse Conv3d (3x3x3 kmap) + BatchNorm + ReLU on 8 TRN2 NeuronCores — v2.

Voxel/data parallel per the sharding hint: output voxels sharded 15000/core.
Off-center offsets use a per-core compacted bf16 source table (unique halo+
local sources, <32767 rows so indices fit int16 in ONE bank) and the
dma_gather transpose path: each gathered 256B token IS a matmul lhsT column
([64 cin on partitions 0:64, zeros 64:128]), so chunks of 128 tokens feed
  matmul(out=[128 tok, 64 cout], lhsT=gathered[:, chunk], rhs=Wk_stack)
directly — no PE transposes, no per-chunk fixup copies. Results are cast to
bf16 and dma_scatter_add'ed (parity-split SBUF CCE) into one of 4 accumulator
pairs; the 26 offsets are split into 4 groups on 4 SWDGE queues so the
scatter chains run concurrently. The center offset (identity map) is a plain
transposed matmul over a host-transposed bf16 slice, initializing pair 0.
BN stats come from ones/X^T X matmuls on the combined accumulator, an
AllReduce over the 8 cores, then an in-place affine+ReLU and bf16 output
(host upcasts to fp32).
"""

import sys
import os

for _p in ("/opt/trn_rl_repo", "/root/.axon_site/_ro/trn_rl_repo"):
    if os.path.isdir(_p) and _p not in sys.path:
        sys.path.insert(0, _p)

import numpy as np

N = 120000
CIN = 64
COUT = 64
K = 27
CENTER = 13
EPS = 1e-5
NCORES = 8
NC_ROWS = N // NCORES          # 15000
SLOTS = 118                    # ceil(15000/128); wrapped rows = 15104
WRAP_ROWS = SLOTS * 128        # 15104
TRASH = WRAP_ROWS - 1          # trash dst row (only ever receives zeros)
HGRP = (SLOTS + 1) // 2        # 59 groups per parity
NQ = 4                         # scatter groups == SWDGE queues


def _wrap16(idx):
    """Wrap an int stream into the [128, n/16] int16 layout dma_gather expects."""
    n = len(idx)
    assert n % 16 == 0
    w = np.ascontiguousarray(idx.reshape(n // 16, 16).T).astype(np.int16)
    return np.tile(w, (8, 1))


def _plan(nbr):
    """Host-side index preprocessing.

    Static (shared) metadata: per-offset chunk counts CK (max over cores),
    chunk->offset map, group split. Per-core: gather/scatter int16 streams and
    the local source row list for the compacted table."""
    offs = [k for k in range(K) if k != CENTER]
    pairs = {}                  # (c, k) -> (src_global, dst_local)
    cnt = np.zeros((NCORES, K), np.int64)
    for k in offs:
        v = nbr[k]
        for c in range(NCORES):
            seg = v[c * NC_ROWS:(c + 1) * NC_ROWS]
            val = np.nonzero(seg >= 0)[0]
            pairs[(c, k)] = (seg[val].astype(np.int64), val)
            cnt[c, k] = len(val)
    CK = {k: int(-(-cnt[:, k].max() // 128)) for k in offs}
    CK_tot = sum(CK.values())
    T_total = CK_tot * 128

    # split offsets into NQ groups balanced by chunk count
    order = sorted(offs, key=lambda k: -CK[k])
    groups = [[] for _ in range(NQ)]
    gload = [0] * NQ
    for k in order:
        g = int(np.argmin(gload))
        groups[g].append(k)
        gload[g] += CK[k]
    # keep original k order within groups (deterministic)
    groups = [sorted(g) for g in groups]
    # chunk layout: group-major, then k in group order
    k_seq = [k for g in groups for k in g]
    ck0 = {}
    p = 0
    for k in k_seq:
        ck0[k] = p
        p += CK[k]
    grp_tok0 = []
    grp_ntok = []
    p = 0
    for g in groups:
        grp_tok0.append(p * 128)
        ng = sum(CK[k] for k in g) * 128
        grp_ntok.append(ng)
        p += sum(CK[k] for k in g)

    # per-core local source tables + streams
    lt_rows = 0
    srcs_cores, gidx_cores, sidx_cores = [], [], []
    for c in range(NCORES):
        allsrc = np.concatenate([pairs[(c, k)][0] for k in offs])
        uniq = np.unique(allsrc)
        srcs_cores.append(uniq)
        lt_rows = max(lt_rows, len(uniq))
    LT = lt_rows + 1            # final row = zeros
    ZROW = LT - 1
    assert LT <= 32767, LT

    for c in range(NCORES):
        uniq = srcs_cores[c]
        gstream = np.full(T_total, ZROW, np.int64)
        sstream = np.full(T_total, TRASH, np.int64)
        for k in k_seq:
            src, dst = pairs[(c, k)]
            base = ck0[k] * 128
            loc = np.searchsorted(uniq, src)
            gstream[base:base + len(src)] = loc
            # dst (local row id) -> wrapped row id == same numbering (row r
            # of the core slice sits at wrapped position r)
            sstream[base:base + len(dst)] = dst
        gidx_cores.append(_wrap16(gstream))
        sidx_cores.append(_wrap16(sstream))

    meta = dict(offs=offs, CK=CK, CK_tot=CK_tot, T_total=T_total, LT=LT,
                groups=groups, k_seq=k_seq, ck0=ck0,
                grp_tok0=grp_tok0, grp_ntok=grp_ntok)
    return meta, gidx_cores, sidx_cores, srcs_cores


def _build_bass(meta):
    from concourse import mybir, bacc
    import concourse.tile as tile
    from concourse.masks import make_identity

    CK = meta["CK"]
    CK_tot = meta["CK_tot"]
    T_total = meta["T_total"]
    LT = meta["LT"]
    groups = meta["groups"]
    k_seq = meta["k_seq"]
    ck0 = meta["ck0"]
    grp_tok0 = meta["grp_tok0"]
    grp_ntok = meta["grp_ntok"]
    f32 = mybir.dt.float32
    bf16 = mybir.dt.bfloat16
    i16 = mybir.dt.int16
    offs = meta["offs"]

    nc = bacc.Bacc("TRN2", target_bir_lowering=False, debug=False,
                   num_devices=NCORES, num_swdge_queues=1)
    lt = nc.dram_tensor("lt", [LT, 128], bf16, kind="ExternalInput").ap()
    ftc = nc.dram_tensor("ftc", [CIN, WRAP_ROWS], bf16,
                         kind="ExternalInput").ap()
    wst = nc.dram_tensor("wst", [128, len(offs) * COUT], bf16,
                         kind="ExternalInput").ap()
    wc = nc.dram_tensor("wc", [CIN, COUT], bf16, kind="ExternalInput").ap()
    gidx = nc.dram_tensor("gidx", [128, T_total // 16], i16,
                          kind="ExternalInput").ap()
    sixd = nc.dram_tensor("sixd", [128, T_total // 16], i16,
                          kind="ExternalInput").ap()
    gbeta = nc.dram_tensor("gbeta", [1, 128], f32, kind="ExternalInput").ap()
    oute = nc.dram_tensor("oute", [128, HGRP, COUT], bf16,
                          kind="ExternalOutput").ap()
    outo = nc.dram_tensor("outo", [128, HGRP, COUT], bf16,
                          kind="ExternalOutput").ap()

    # offset -> column in wst
    kcol = {k: i for i, k in enumerate(offs)}

    with tile.TileContext(nc) as tc:
        with tc.tile_pool(name="sb", bufs=1) as pool, \
             tc.tile_pool(name="ps", bufs=2, space="PSUM") as psum, \
             tc.tile_pool(name="dram", bufs=1, space="DRAM") as dram:
            ident = pool.tile([128, 128], f32)
            make_identity(nc, ident[:])
            ones_b = pool.tile([128, 1], bf16)
            nc.vector.memset(ones_b[:], 1.0)
            onesr = pool.tile([1, 128], f32)
            nc.vector.memset(onesr[:], 1.0)
            istack = pool.tile([128, COUT], f32)
            nc.vector.tensor_copy(out=istack[0:64, :], in_=ident[0:64, 0:64])
            nc.vector.tensor_copy(out=istack[64:128, :],
                                  in_=ident[64:128, 64:128])

            gix = pool.tile([128, T_total // 16], i16)
            nc.sync.dma_start(out=gix[:], in_=gidx[:])
            six = pool.tile([128, T_total // 16], i16)
            nc.sync.dma_start(out=six[:], in_=sixd[:])
            wsb = pool.tile([128, len(offs) * COUT], bf16)
            nc.sync.dma_start(out=wsb[:], in_=wst[:])
            wcb = pool.tile([CIN, COUT], bf16)
            nc.sync.dma_start(out=wcb[:], in_=wc[:])
            gb = pool.tile([1, 128], f32)
            nc.sync.dma_start(out=gb[:], in_=gbeta[:])
            fts = pool.tile([CIN, WRAP_ROWS], bf16)
            nc.sync.dma_start(out=fts[:], in_=ftc[:])

            # 4 accumulator pairs (bf16). Pair 0 is initialized by the center
            # pass; pairs 1..3 are zeroed.
            aes = [pool.tile([128, HGRP, COUT], bf16, tag=f"ae{g}",
                             name=f"ae{g}") for g in range(NQ)]
            aos = [pool.tile([128, HGRP, COUT], bf16, tag=f"ao{g}",
                             name=f"ao{g}") for g in range(NQ)]
            for g in range(1, NQ):
                nc.scalar.memzero(aes[g][:])
                nc.scalar.memzero(aos[g][:])

            # ---- gathers: one per group, on its own SWDGE queue ----
            gths = []
            for g in range(NQ):
                gt = pool.tile([128, 1, grp_ntok[g]], bf16, tag=f"g{g}",
                               name=f"gth{g}")
                gths.append(gt)
                nc.gpsimd.dma_gather(
                    out_ap=gt[:], in_ap=lt[:],
                    idxs_ap=gix[:, grp_tok0[g] // 16:
                                (grp_tok0[g] + grp_ntok[g]) // 16],
                    num_idxs=grp_ntok[g], num_idxs_reg=grp_ntok[g],
                    elem_size=128, transpose=True, single_packet=False)

            # ---- center pass: matmul ftc columns with wc, init ae0/ao0 ----
            for j0 in range(0, SLOTS, 8):
                jn = min(8, SLOTS - j0)
                ne = (jn + 1) // 2
                no = jn // 2
                pc = psum.tile([128, 8, COUT], f32, tag="pc")
                for j in range(j0, j0 + jn):
                    lhsT = fts[:, j * 128:(j + 1) * 128]
                    if j % 2 == 0:
                        out_ap = pc[:, (j - j0) // 2, :]
                    else:
                        out_ap = pc[:, 4 + (j - j0) // 2, :]
                    nc.tensor.matmul(out=out_ap, lhsT=lhsT, rhs=wcb[:],
                                     start=True, stop=True)
                g0 = j0 // 2
                eng = nc.vector if (j0 // 8) % 2 == 0 else nc.scalar
                if eng is nc.vector:
                    nc.vector.tensor_copy(out=aes[0][:, g0:g0 + ne, :],
                                          in_=pc[:, 0:ne, :])
                    if no:
                        nc.vector.tensor_copy(out=aos[0][:, g0:g0 + no, :],
                                              in_=pc[:, 4:4 + no, :])
                else:
                    nc.scalar.copy(out=aes[0][:, g0:g0 + ne, :],
                                   in_=pc[:, 0:ne, :])
                    if no:
                        nc.scalar.copy(out=aos[0][:, g0:g0 + no, :],
                                       in_=pc[:, 4:4 + no, :])

            # ---- off-center: chunk matmuls -> Y (bf16) -> scatter-add ----
            ybuf = pool.tile([128, CK_tot, COUT], bf16)
            for g in range(NQ):
                gt = gths[g]
                base_ck = ck0[groups[g][0]]
                chunks = []     # (global chunk id, k)
                for k in groups[g]:
                    for j in range(CK[k]):
                        chunks.append((ck0[k] + j, k))
                for i0 in range(0, len(chunks), 8):
                    inb = min(8, len(chunks) - i0)
                    py = psum.tile([128, 8, COUT], f32, tag="py", bufs=3)
                    for q in range(inb):
                        cid, k = chunks[i0 + q]
                        loc = (cid - base_ck) * 128
                        nc.tensor.matmul(
                            out=py[:, q, :],
                            lhsT=gt[:, 0, loc:loc + 128],
                            rhs=wsb[:, kcol[k] * COUT:(kcol[k] + 1) * COUT],
                            start=True, stop=True)
                    c0 = chunks[i0][0]
                    if (i0 // 8) % 2 == 0:
                        nc.vector.tensor_copy(out=ybuf[:, c0:c0 + inb, :],
                                              in_=py[:, 0:inb, :])
                    else:
                        nc.scalar.copy(out=ybuf[:, c0:c0 + inb, :],
                                       in_=py[:, 0:inb, :])
                for k in groups[g]:
                    nc.gpsimd.dma_scatter_add(
                        out_ap=aes[g][:], in_ap=ybuf[:, ck0[k]:ck0[k] + CK[k], :],
                        idxs_ap=six[:, ck0[k] * 8:(ck0[k] + CK[k]) * 8],
                        num_idxs=CK[k] * 128, num_idxs_reg=CK[k] * 128,
                        elem_size=COUT, sbuf_tokens_per_rank=128,
                        parity_reg=0, out_ap_other=aos[g][:],
                        single_packet=False)

            # ---- combine pairs into pair 0 ----
            nc.vector.tensor_add(out=aes[1][:], in0=aes[1][:], in1=aes[2][:])
            nc.vector.tensor_add(out=aos[1][:], in0=aos[1][:], in1=aos[2][:])
            nc.vector.tensor_add(out=aes[0][:], in0=aes[0][:], in1=aes[3][:])
            nc.vector.tensor_add(out=aos[0][:], in0=aos[0][:], in1=aos[3][:])
            nc.vector.tensor_add(out=aes[0][:], in0=aes[0][:], in1=aes[1][:])
            nc.vector.tensor_add(out=aos[0][:], in0=aos[0][:], in1=aos[1][:])
            ae, ao = aes[0], aos[0]

            # ---- stats: sums + sum-squares over all rows ----
            # order: full-width [128,2,64] slices first and last so every
            # psum element's first write has start semantics and last write
            # carries stop (the [128,1,64] leftovers sit in the middle)
            pcov = psum.tile([128, 128], f32, tag="py", bufs=3)
            cov_ins = []
            for g0 in range(0, HGRP - 1, 2):
                cov_ins.append(ae[:, g0:g0 + 2, :])
            cov_ins.append(ae[:, HGRP - 1:HGRP, :])
            cov_ins.append(ao[:, HGRP - 1:HGRP, :])
            for g0 in range(0, HGRP - 1, 2):
                cov_ins.append(ao[:, g0:g0 + 2, :])
            for i, ap in enumerate(cov_ins):
                w = ap.shape[1] * COUT
                nc.tensor.matmul(out=pcov[0:w, 0:w], lhsT=ap, rhs=ap,
                                 start=(i == 0), stop=(i == len(cov_ins) - 1))
            psumr = psum.tile([1, 512], f32, tag="pc")
            sum_ins = []
            for g0 in range(0, HGRP - 8, 8):
                sum_ins.append(ae[:, g0:g0 + 8, :])
            sum_ins.append(ae[:, HGRP - (HGRP % 8 or 8):HGRP, :])
            sum_ins.append(ao[:, HGRP - (HGRP % 8 or 8):HGRP, :])
            for g0 in range(0, HGRP - 8, 8):
                sum_ins.append(ao[:, g0:g0 + 8, :])
            for i, ap in enumerate(sum_ins):
                w = ap.shape[1] * COUT
                nc.tensor.matmul(out=psumr[:, 0:w], lhsT=ones_b[:], rhs=ap,
                                 start=(i == 0), stop=(i == len(sum_ins) - 1))
            tmpc = pool.tile([128, 128], f32)
            nc.vector.tensor_mul(out=tmpc[:], in0=pcov[:], in1=ident[:])
            diagc = pool.tile([128, 1], f32)
            nc.vector.tensor_reduce(out=diagc[:], in_=tmpc[:],
                                    axis=mybir.AxisListType.X,
                                    op=mybir.AluOpType.add)
            psq = psum.tile([1, COUT], f32, tag="pq")
            nc.tensor.matmul(out=psq[:], lhsT=diagc[:], rhs=istack[:],
                             start=True, stop=True)
            ssum = pool.tile([1, 512], f32)
            nc.vector.tensor_copy(out=ssum[:], in_=psumr[:])
            nc.vector.tensor_add(out=ssum[:, 0:256], in0=ssum[:, 0:256],
                                 in1=ssum[:, 256:512])
            nc.vector.tensor_add(out=ssum[:, 0:128], in0=ssum[:, 0:128],
                                 in1=ssum[:, 128:256])
            nc.vector.tensor_add(out=ssum[:, 0:64], in0=ssum[:, 0:64],
                                 in1=ssum[:, 64:128])
            stats = pool.tile([1, 128], f32)
            nc.vector.tensor_copy(out=stats[:, 0:64], in_=ssum[:, 0:64])
            nc.vector.tensor_copy(out=stats[:, 64:128], in_=psq[:])

            # ---- AllReduce over 8 cores ----
            cin_d = dram.tile([1, 128], f32)
            cout_d = dram.tile([1, 128], f32)
            nc.sync.dma_start(out=cin_d[:], in_=stats[:])
            if os.environ.get("BASS_SIM_NO_COLLECTIVE"):
                nc.sync.dma_start(out=cout_d[:], in_=cin_d[:])
            else:
                nc.gpsimd.collective_compute(
                    "AllReduce", mybir.AluOpType.add,
                    replica_groups=[list(range(NCORES))],
                    ins=[cin_d.opt()], outs=[cout_d.opt()])
            red = pool.tile([1, 128], f32)
            nc.sync.dma_start(out=red[:], in_=cout_d[:])

            # ---- affine params ----
            nscale = 1.0 / N
            if os.environ.get("BASS_SIM_NO_COLLECTIVE"):
                nscale = 1.0 / NC_ROWS
            mean = pool.tile([1, COUT], f32)
            nc.vector.tensor_scalar_mul(out=mean[:], in0=red[:, 0:64],
                                        scalar1=nscale)
            ex2 = pool.tile([1, COUT], f32)
            nc.vector.tensor_scalar_mul(out=ex2[:], in0=red[:, 64:128],
                                        scalar1=nscale)
            var = pool.tile([1, COUT], f32)
            nc.vector.tensor_mul(out=var[:], in0=mean[:], in1=mean[:])
            nc.vector.tensor_sub(out=var[:], in0=ex2[:], in1=var[:])
            nc.vector.tensor_scalar_add(out=var[:], in0=var[:], scalar1=EPS)
            std = pool.tile([1, COUT], f32)
            nc.scalar.sqrt(out=std[:], in_=var[:])
            rstd = pool.tile([1, COUT], f32)
            nc.vector.reciprocal(out=rstd[:], in_=std[:])
            scl = pool.tile([1, COUT], f32)
            nc.vector.tensor_mul(out=scl[:], in0=gb[:, 0:64], in1=rstd[:])
            bia = pool.tile([1, COUT], f32)
            nc.vector.tensor_mul(out=bia[:], in0=mean[:], in1=scl[:])
            nc.vector.tensor_sub(out=bia[:], in0=gb[:, 64:128], in1=bia[:])

            # broadcast to [128, 8, 64] bf16
            pbs = psum.tile([128, COUT], f32, tag="pq")
            nc.tensor.matmul(out=pbs[:], lhsT=onesr[:], rhs=scl[:],
                             start=True, stop=True)
            s8 = pool.tile([128, 8, COUT], bf16)
            nc.vector.tensor_copy(out=s8[:, 0, :], in_=pbs[:])
            pbb = psum.tile([128, COUT], f32, tag="pq")
            nc.tensor.matmul(out=pbb[:], lhsT=onesr[:], rhs=bia[:],
                             start=True, stop=True)
            b8 = pool.tile([128, 8, COUT], bf16)
            nc.vector.tensor_copy(out=b8[:, 0, :], in_=pbb[:])
            for t8 in (s8, b8):
                nc.vector.tensor_copy(out=t8[:, 1:2, :], in_=t8[:, 0:1, :])
                nc.vector.tensor_copy(out=t8[:, 2:4, :], in_=t8[:, 0:2, :])
                nc.vector.tensor_copy(out=t8[:, 4:8, :], in_=t8[:, 0:4, :])

            # ---- normalize + relu in place, then write out ----
            for t in (ae, ao):
                for g0 in range(0, HGRP, 8):
                    gn = min(8, HGRP - g0)
                    sl = t[:, g0:g0 + gn, :]
                    nc.vector.tensor_mul(out=sl, in0=sl, in1=s8[:, 0:gn, :])
                    nc.vector.tensor_add(out=sl, in0=sl, in1=b8[:, 0:gn, :])
                    nc.vector.tensor_scalar_max(out=sl, in0=sl, scalar1=0.0)
            nc.sync.dma_start(out=oute[:], in_=ae[:, :, :])
            nc.sync.dma_start(out=outo[:], in_=ao[:, :, :])

    nc.compile()
    return nc


def _host_tensors(feats, weight, gamma, beta, meta, srcs_cores):
    import ml_dtypes
    bf = ml_dtypes.bfloat16
    feats = np.ascontiguousarray(np.asarray(feats, dtype=np.float32))
    f16 = feats.astype(bf)
    weight = np.asarray(weight, dtype=np.float32)
    offs = meta["offs"]
    LT = meta["LT"]

    wstack = np.zeros((128, len(offs) * COUT), np.float32)
    for i, k in enumerate(offs):
        wstack[0:CIN, i * COUT:(i + 1) * COUT] = weight[k]
    wstack = wstack.astype(bf)
    wcv = weight[CENTER].astype(bf)

    gbv = np.zeros((1, 128), np.float32)
    gbv[0, 0:64] = np.asarray(gamma, np.float32)
    gbv[0, 64:128] = np.asarray(beta, np.float32)

    lts, ftcs = [], []
    for c in range(NCORES):
        t = np.zeros((LT, 128), bf)
        u = srcs_cores[c]
        t[:len(u), 0:CIN] = f16[u]
        lts.append(t)
        ft = np.zeros((CIN, WRAP_ROWS), bf)
        ft[:, :NC_ROWS] = f16[c * NC_ROWS:(c + 1) * NC_ROWS].T
        ftcs.append(ft)
    return lts, ftcs, wstack, wcv, gbv


def _prepare(np_inputs):
    nbr = np.asarray(np_inputs["neighbor_idx"])
    meta, gidx_cores, sidx_cores, srcs_cores = _plan(nbr)
    nc = _build_bass(meta)
    lts, ftcs, wstack, wcv, gbv = _host_tensors(
        np_inputs["feats"], np_inputs["weight"], np_inputs["gamma"],
        np_inputs["beta"], meta, srcs_cores)
    in_maps = [
        {"lt": lts[c], "ftc": ftcs[c], "wst": wstack, "wc": wcv,
         "gidx": gidx_cores[c], "sixd": sidx_cores[c], "gbeta": gbv}
        for c in range(NCORES)
    ]
    return nc, in_maps


def kernel(feats, weight, gamma, beta, neighbor_idx):
    from concourse.bass_utils import run_bass_kernel_spmd

    np_inputs = {"feats": feats, "weight": weight, "gamma": gamma,
                 "beta": beta, "neighbor_idx": neighbor_idx}
    nc, in_maps = _prepare(np_inputs)
    res = run_bass_kernel_spmd(nc, in_maps, core_ids=list(range(NCORES)))
    out = np.empty((N, COUT), np.float32)
    for c in range(NCORES):
        wrapped = np.empty((128, SLOTS, COUT), np.float32)
        wrapped[:, 0::2, :] = res.results[c]["oute"].astype(np.float32)
        wrapped[:, 1::2, :] = res.results[c]["outo"].astype(np.float32)
        rows = wrapped.transpose(1, 0, 2).reshape(WRAP_ROWS, COUT)
        out[c * NC_ROWS:(c + 1) * NC_ROWS] = rows[:NC_ROWS]
    return out
